# revision 19
# baseline (speedup 1.0000x reference)
"""MoE routing gate kernel for Trainium2 (8 NeuronCores, data-parallel).

Computes, for x[32768, 2048], weight[64, 2048], bias[64]:
    logits = x @ weight.T
    probs  = softmax(logits, axis=-1)
    idx    = top_k(probs + bias, 6).indices
    w      = take_along_axis(probs, idx)
returning (w float32 [32768, 6], idx int32 [32768, 6]).

Sharding: tokens split 4096/core across 8 cores; weight/bias replicated.

Memory-bound: the floor is streaming x.  x moves at 3 bytes/element:
fp16 hi (2B) + e4m3 of the residual scaled by 2^16 (1B).  Logits come
from 3 PE passes accumulating fp32 in one PSUM bank:
    x16*w16hi + x16*w16lo + xlo8*(w*2^-16 in bf16)
(the fp8 rhs / bf16 lhsT pass is legal - both upconvert to FP22 - so
the 2^-16 scale folds into the stationary operand).  Measured rel err
vs the fp32 reference: ~4e-3 on indices / 2e-3 on weights (top-k tie
noise floor), against the 2e-2 gate.

Per-core pipeline (25.2MB shard, ~64us DMA floor at ~390 GB/s):
  - 3 DMAs of 2MB per 1024-token super-group (two fp16 halves + fp8
    lo), 16KB contiguous per partition, on the sync HWDGE queue.
  - Matmul pairs column-tiled: g=0 in PE cols 0-63 (PSUM part 0-63),
    g=1 in cols 64-127, streaming concurrently.
  - logits^T -> ACT copy to SBUF -> per-j PE transpose (identity
    matmul) -> per-j ACT exp with accum_out sum -> fused DVE stt
    selection key q = exp + sum*bias -> DVE Max8/MaxIndex8.
  - Gather of the 6 unbiased weights via a batched one-hot in bf16:
    is_equal [128,8,6,64], multiply by exp (bf16 copy), X-axis reduce,
    scale by 1/sum; staged with indices in one [128,96] tile -> single
    48KB DMA per super-group on the scalar HWDGE queue.
"""

import numpy as np
import ml_dtypes

import concourse.bacc as bacc
import concourse.bass as bass
import concourse.mybir as mybir
import concourse.tile as tile
from concourse.bass_utils import run_bass_kernel_spmd

BF16 = mybir.dt.bfloat16
F16 = mybir.dt.float16
F8E4 = mybir.dt.float8e4
F32 = mybir.dt.float32
I32 = mybir.dt.int32
U32 = mybir.dt.uint32
AX = mybir.AxisListType.X
OP = mybir.AluOpType
EXP = mybir.ActivationFunctionType.Exp

TOKENS, DIM, E, TOPK, NCORES = 32768, 2048, 64, 6, 8
KC = DIM // 128   # 16 contraction chunks of 128
KH = KC // 2      # k-chunks per fp16 DMA half
LO_SCALE = 2.0 ** 16


def build_nc(tpc, sg_t=1024):
    """Build the per-core Bass program for a tpc-token shard."""
    grp = sg_t // 2          # tokens per matmul (N), two col-tiled groups
    assert grp == 512
    nsg = tpc // sg_t
    nj = sg_t // 128         # 128-token tiles per super-group
    cols = nj * TOPK

    nc = bacc.Bacc("TRN2", target_bir_lowering=False, debug=False)

    xh = nc.dram_tensor("xh", [nsg, 2, 128, KH, sg_t], F16, kind="ExternalInput")
    xl = nc.dram_tensor("xl", [nsg, 128, KC, sg_t], F8E4, kind="ExternalInput")
    wt_hi = nc.dram_tensor("wt_hi", [128, KC, E], F16, kind="ExternalInput")
    wt_lo = nc.dram_tensor("wt_lo", [128, KC, E], F16, kind="ExternalInput")
    wt_c = nc.dram_tensor("wt_c", [128, KC, E], BF16, kind="ExternalInput")
    bias_b = nc.dram_tensor("bias_b", [128, E], F32, kind="ExternalInput")
    iota_bf = nc.dram_tensor("iota_bf", [128, E], BF16, kind="ExternalInput")
    ident2 = nc.dram_tensor("ident2", [128, 64], F32, kind="ExternalInput")
    out_all = nc.dram_tensor("out_all", [nsg, 128, 2 * cols], F32,
                             kind="ExternalOutput")

    with tile.TileContext(nc) as tc:
        with (
            tc.tile_pool(name="consts", bufs=1) as cpool,
            tc.tile_pool(name="xhbuf", bufs=3) as xhp,
            tc.tile_pool(name="xlbuf", bufs=3) as xlp,
            tc.tile_pool(name="lt", bufs=3) as ltp,
            tc.tile_pool(name="ex", bufs=3) as exp_,
            tc.tile_pool(name="small", bufs=4) as smp,
            tc.tile_pool(name="work", bufs=3) as wkp,
            tc.tile_pool(name="stage", bufs=3) as stp,
            tc.tile_pool(name="acc", bufs=3, space="PSUM") as accp,
            tc.tile_pool(name="tr", bufs=5, space="PSUM") as trp,
        ):
            cwh = cpool.tile([128, KC, E], F16)
            nc.scalar.dma_start(cwh, wt_hi[:])
            cwl = cpool.tile([128, KC, E], F16)
            nc.scalar.dma_start(cwl, wt_lo[:])
            cwc = cpool.tile([128, KC, E], BF16)
            nc.scalar.dma_start(cwc, wt_c[:])
            cbias = cpool.tile([128, E], F32)
            nc.scalar.dma_start(cbias, bias_b[:])
            ciota = cpool.tile([128, E], BF16)
            nc.scalar.dma_start(ciota, iota_bf[:])
            cident = cpool.tile([128, 64], F32)
            nc.scalar.dma_start(cident, ident2[:])

            xl_hoist = {}
            for sg in range(nsg):
                last = sg == nsg - 1
                xh0 = xhp.tile([128, KH, sg_t], F16, tag="xh")
                nc.sync.dma_start(xh0, xh[sg, 0])
                xh1 = xhp.tile([128, KH, sg_t], F16, tag="xh")
                nc.sync.dma_start(xh1, xh[sg, 1])
                if last:
                    # fp8 chunk already DMA'd (hoisted during sg-1): its
                    # C-pass matmuls run first, off the post-DMA tail.
                    xlo = xl_hoist.pop(sg)
                else:
                    if sg == nsg - 2:
                        xln = xlp.tile([128, KC, sg_t], F8E4, tag="xl")
                        nc.sync.dma_start(xln, xl[nsg - 1])
                        xl_hoist[nsg - 1] = xln
                    xlo = xlp.tile([128, KC, sg_t], F8E4, tag="xl")
                    nc.sync.dma_start(xlo, xl[sg])

                # 96 matmul pair-slots into one accumulator: 2 fp16 passes
                # (w hi, w lo) + 1 fp8 residual pass.
                acc = accp.tile([128, grp], F32)

                def mm_pair(w_ap, x_ap, first, last_mm):
                    nc.tensor.matmul(
                        acc[0:64], w_ap, x_ap[:, 0:grp],
                        start=first, stop=last_mm, tile_position=(0, 0),
                    )
                    nc.tensor.matmul(
                        acc[64:128], w_ap, x_ap[:, grp:sg_t],
                        start=first, stop=last_mm, tile_position=(0, 64),
                        skip_group_check=True,
                    )

                if last:
                    for k in range(KC):
                        mm_pair(cwc[:, k, :], xlo[:, k], k == 0, False)
                    for h, xht in ((0, xh0), (1, xh1)):
                        for k8 in range(KH):
                            k = KH * h + k8
                            xk = xht[:, k8]
                            mm_pair(cwh[:, k, :], xk, False, False)
                            mm_pair(cwl[:, k, :], xk, False,
                                    h == 1 and k8 == KH - 1)
                else:
                    for h, xht in ((0, xh0), (1, xh1)):
                        for k8 in range(KH):
                            k = KH * h + k8
                            xk = xht[:, k8]
                            mm_pair(cwh[:, k, :], xk, h == 0 and k8 == 0, False)
                            mm_pair(cwl[:, k, :], xk, False, False)
                    for k in range(KC):
                        mm_pair(cwc[:, k, :], xlo[:, k], False, k == KC - 1)

                lt = ltp.tile([128, grp], F32)
                nc.scalar.copy(lt, acc)

                tps = []
                for j in range(nj):
                    base = 64 * (j // 4)
                    tpj = trp.tile([128, 64], F32, tag="tps")
                    nc.tensor.transpose(
                        tpj,
                        lt[base:base + 64, (j % 4) * 128:(j % 4 + 1) * 128],
                        cident[base:base + 64, :],
                    )
                    tps.append(tpj)

                ex = exp_.tile([128, nj, E], F32, tag="ex")
                ssum = smp.tile([128, nj], F32, tag="ssum")
                q = wkp.tile([128, nj, E], F32, tag="q")
                mx = smp.tile([128, nj, 8], F32, tag="mx")
                mi = smp.tile([128, nj, 8], U32, tag="mi")
                for j in range(nj):
                    nc.scalar.activation(
                        ex[:, j], tps[j], EXP,
                        accum_out=ssum[:, j:j + 1],
                    )
                    nc.vector.scalar_tensor_tensor(
                        q[:, j], cbias, ssum[:, j:j + 1], ex[:, j],
                        OP.mult, OP.add,
                    )
                    nc.vector.max(mx[:, j], q[:, j])
                    nc.vector.max_index(mi[:, j], mx[:, j], q[:, j])

                exb = exp_.tile([128, nj, E], BF16, tag="exb")
                nc.scalar.copy(exb, ex)
                rs = smp.tile([128, nj], F32, tag="rs")
                nc.vector.reciprocal(rs, ssum)
                idxb = smp.tile([128, nj, 8], BF16, tag="idxb")
                nc.vector.tensor_copy(idxb, mi)

                oh = wkp.tile([128, nj, TOPK, E], BF16, tag="oh")
                nc.vector.tensor_tensor(
                    oh,
                    idxb[:, :, 0:TOPK].unsqueeze(3).broadcast_to(
                        [128, nj, TOPK, E]),
                    ciota.unsqueeze(1).unsqueeze(1).broadcast_to(
                        [128, nj, TOPK, E]),
                    OP.is_equal,
                )
                ohx = wkp.tile([128, nj, TOPK, E], BF16, tag="ohx")
                nc.vector.tensor_tensor(
                    ohx, oh,
                    exb.unsqueeze(2).broadcast_to([128, nj, TOPK, E]),
                    OP.mult,
                )
                g6 = smp.tile([128, nj, TOPK], F32, tag="g6")
                nc.vector.tensor_reduce(g6, ohx, AX, OP.add)

                stage = stp.tile([128, 2 * cols], F32, tag="st")
                st_w = stage[:, 0:cols].rearrange(
                    "p (j k) -> p j k", j=nj, k=TOPK)
                st_i = stage[:, cols:2 * cols].bitcast(U32).rearrange(
                    "p (j k) -> p j k", j=nj, k=TOPK)
                nc.vector.tensor_tensor(
                    st_w, g6,
                    rs.unsqueeze(2).broadcast_to([128, nj, TOPK]),
                    OP.mult,
                )
                nc.vector.tensor_copy(st_i, mi[:, :, 0:TOPK])
                nc.scalar.dma_start(out_all[sg], stage)
    return nc


_CACHE = {}


def _get_compiled(tpc):
    if tpc not in _CACHE:
        nc = build_nc(tpc)
        nc.compile()
        _CACHE[tpc] = nc
    return _CACHE[tpc]


def _prep_shared(weight, bias):
    f16 = np.float16
    bf = ml_dtypes.bfloat16
    w = np.asarray(weight, np.float32)
    w_hi = w.astype(f16)
    w_lo = (w - w_hi.astype(np.float32)).astype(f16)
    w_c = (w * (1.0 / LO_SCALE)).astype(bf)
    iota = np.arange(E, dtype=np.float32)

    def wtile(a):  # [E, DIM] -> [128, KC, E]
        return np.ascontiguousarray(
            np.ascontiguousarray(a.T).reshape(KC, 128, E).transpose(1, 0, 2)
        )

    return {
        "wt_hi": wtile(w_hi),
        "wt_lo": wtile(w_lo),
        "wt_c": wtile(w_c),
        "bias_b": np.ascontiguousarray(
            np.broadcast_to(np.asarray(bias, np.float32), (128, E))
        ),
        "iota_bf": np.ascontiguousarray(np.broadcast_to(iota.astype(bf), (128, E))),
        "ident2": np.ascontiguousarray(
            np.tile(np.eye(64, dtype=np.float32), (2, 1))
        ),
    }


def prep_core_inputs(x, weight, bias, ncores=NCORES, sg_t=1024):
    e4 = ml_dtypes.float8_e4m3
    shared = _prep_shared(weight, bias)
    x = np.asarray(x, np.float32)
    tpc = x.shape[0] // ncores
    nsg = tpc // sg_t
    in_maps = []
    for c in range(ncores):
        xs = np.ascontiguousarray(x[c * tpc:(c + 1) * tpc].T)  # [DIM, tpc]
        xh16 = xs.astype(np.float16)
        xlo = np.clip((xs - xh16.astype(np.float32)) * LO_SCALE, -240, 240)
        xh6 = xh16.reshape(2, KH, 128, nsg, sg_t)
        xh_pk = np.ascontiguousarray(xh6.transpose(3, 0, 2, 1, 4))
        xl6 = xlo.astype(e4).reshape(KC, 128, nsg, sg_t)
        xl_pk = np.ascontiguousarray(xl6.transpose(2, 1, 0, 3))
        in_maps.append({"xh": xh_pk, "xl": xl_pk, **shared})
    return in_maps


def unpack_outputs(res_list, tpc):
    ws, idxs = [], []
    for r in res_list:
        ov = np.asarray(r["out_all"])  # [nsg, 128, 2*cols]
        nsg = ov.shape[0]
        cols = ov.shape[2] // 2
        wv = ov[:, :, 0:cols]
        iv = ov[:, :, cols:].view(np.int32)
        wv = wv.reshape(nsg, 128, -1, TOPK).transpose(0, 2, 1, 3).reshape(tpc, TOPK)
        iv = iv.reshape(nsg, 128, -1, TOPK).transpose(0, 2, 1, 3).reshape(tpc, TOPK)
        ws.append(wv)
        idxs.append(iv)
    return (
        np.ascontiguousarray(np.concatenate(ws)).astype(np.float32),
        np.ascontiguousarray(np.concatenate(idxs)).astype(np.int32),
    )


def run(x, weight, bias, trace=False, **kwargs):
    x = np.asarray(x, np.float32)
    tpc = x.shape[0] // NCORES
    nc = _get_compiled(tpc)
    in_maps = prep_core_inputs(x, weight, bias)
    res = run_bass_kernel_spmd(nc, in_maps, list(range(NCORES)), trace=trace, **kwargs)
    w, i = unpack_outputs(res.results, tpc)
    return w, i, res


def kernel(x, weight, bias):
    w, i, _ = run(x, weight, bias, trace=False)
    return w, i
